# revision 2
# baseline (speedup 1.0000x reference)
"""DiffusionGraphConv (3-hop symmetric-normalized diffusion + linear) on 8 TRN2 cores.

Math (reference):
    deg  = segment_sum(1, dst); norm = clip(deg,1)^-0.5
    h_0  = feat
    h_k  = norm * segment_sum(norm[src] * h_{k-1}[src] -> dst)
    out  = concat(h_0..h_3) @ W.T + b

Reformulation (all linear; norms folded into per-edge/per-node scalings):
    g_0 = norm * feat              (precomputed on HOST into the gather table)
    s_k = segment_sum(g_{k-1}[src] -> dst)     # pure gather + one-hot matmul
    h_k = norm * s_k ; g_k = norm^2 * s_k ; out = sum_k h_k @ Wk.T + feat@W0.T + b

Distribution: nodes (and their edges, by dst) sharded across 8 cores.
Per hop each core:
  - 14 large dma_gathers (7 window-groups x 2 src-regions, ~8k rows each)
    fetch per-edge-slot source rows (f32, 256B) from the DRAM g-table.
  - One batched bf16 is_equal per (group, region) builds all one-hot S
    blocks at once (stride-0 broadcast APs) -- no per-block DVE ops.
  - ACT converts gathered rows to bf16; 18 bf16 matmuls per window
    segment-sum into PSUM; norm applied via per-partition scalars
    (ncol on DVE for h, ncol^2 on ACT for the exchanged g).
  - Region A (28 windows) AllGathers mid-hop (Shared pair-HBM output);
    region B at hop end.
Final linear is data-parallel in bf16 with replicated W.
"""

import math
import sys

sys.path.insert(0, "/opt/trn_rl_repo")

import numpy as np

import concourse.bacc as bacc
import concourse.mybir as mybir
import concourse.tile as tile
import concourse.tile_sem_assignment as _tsa
from concourse.bass_utils import run_bass_kernel_spmd

# Problem constants (hardcoded per the harness contract).
N = 50000
E = 800000
D = 64
HOPS = 3
NCORES = 8
GRP = 7                      # windows per gather group
SHARD = N // NCORES          # 6250 nodes per core
NWIN = (SHARD + 127) // 128  # 49 windows of 128 nodes
NGRP = NWIN // GRP           # 7 groups
SHARD_PAD = NWIN * 128       # 6272
WA = 25                      # windows in region A (balanced A/B cell sizes)
WB = NWIN - WA               # windows in region B
ROWSA = WA * 128             # 3200 rows per shard in region A
ROWSB = WB * 128             # 3072 rows per shard in region B
REGA = NCORES * ROWSA        # 25600 (int16-safe region sizes)
REGB = NCORES * ROWSB        # 24576
NCELLS = NWIN * 2            # (group, region, win-in-group) cells per core

F32 = mybir.dt.float32
BF16 = mybir.dt.bfloat16
I16 = mybir.dt.int16


def _set_problem(n, e):
    """Recompute derived constants for a different problem size (testing)."""
    global N, E, SHARD, NWIN, NGRP, SHARD_PAD, WA, WB, ROWSA, ROWSB
    global REGA, REGB, NCELLS
    N, E = n, e
    SHARD = N // NCORES
    NWIN = (SHARD + 127) // 128
    assert NWIN % GRP == 0
    NGRP = NWIN // GRP
    SHARD_PAD = NWIN * 128
    WA = (NWIN + 1) // 2
    WB = NWIN - WA
    ROWSA = WA * 128
    ROWSB = WB * 128
    REGA = NCORES * ROWSA
    REGB = NCORES * ROWSB
    NCELLS = NWIN * 2
    assert REGA < 32768 and REGB < 32768


# Timing aid: repeat the computation REPS times inside one NEFF so
# (T(R) - T(1)) / (R - 1) cancels host/dispatch overhead. Leave at 1.
REPS = 1
# Timing-experiment knobs (leave defaults for the graded kernel).
NO_COLLECTIVE = False   # replace AllGathers with local DMA (wrong results)
NO_GATHERS = False      # hops skip dma_gather (compute on stale tiles)
NQUEUES = 4             # SWDGE queues; gathers round-robin across them
SHARED_AG = True        # AllGather outputs in pair-shared HBM
GCH = 0                 # max slots per dma_gather (0 = whole group at once)
SINGLE_PACKET = False   # stream descs (ring can hold < one group)
SCRATCH = 16384         # dynamic_dma_scratch_size (SWDGE desc ring bytes)


# The ucode locks each SWDGE completion semaphore to one queue, but Tile's
# pass-1 lane assigner round-robins Pool DMA instructions across all 8 DMASW
# lanes queue-unaware. Partition the lanes per queue instead: queue q owns
# lanes {q*L .. q*L+L-1}, chosen from the instruction's queue_num.
_orig_assign_tick = _tsa.TileClockTick._assign_tick


def _queue_aware_assign_tick(self, inst):
    q = getattr(inst, "queue_num", None)
    if (
        q is not None
        and inst.engine == mybir.EngineType.Pool
        and isinstance(inst, _tsa.DMAInst)
        and not isinstance(inst, _tsa.bass_isa.UserSyncedRemoteDMADescs)
    ):
        lanes_per_q = max(1, self.swdge_sem_count // max(1, NQUEUES))
        if not hasattr(self, "_q_lane_ctr"):
            self._q_lane_ctr = {}
        c = self._q_lane_ctr.get(q, 0)
        self._q_lane_ctr[q] = c + 1
        self.next_sw_dma_idx = (q % self.swdge_sem_count) * lanes_per_q % (
            self.swdge_sem_count
        ) + (c % lanes_per_q)
    return _orig_assign_tick(self, inst)


_tsa.TileClockTick._assign_tick = _queue_aware_assign_tick


def _preprocess(src, dst):
    """Build per-core gather/segment metadata from the edge list."""
    src = np.asarray(src).astype(np.int64)
    dst = np.asarray(dst).astype(np.int64)

    deg = np.bincount(dst, minlength=N).astype(np.float32)
    norm = np.clip(deg, 1.0, None) ** -0.5

    core = dst // SHARD
    dst_loc = dst - core * SHARD
    win = dst_loc >> 7
    dst_in_win = (dst_loc & 127).astype(np.float32)

    # region-relative gather rows for the source endpoint
    src_r = src // SHARD
    src_i = src % SHARD
    in_b = (src_i >= ROWSA).astype(np.int64)
    rel = np.where(in_b == 0, src_r * ROWSA + src_i, src_r * ROWSB + (src_i - ROWSA))

    # cell = (group, region, win-in-group); cells of one (group, region) are
    # adjacent so one dma_gather covers GRP windows of one source region.
    g = win // GRP
    wi = win % GRP
    cell_l = g * (2 * GRP) + in_b * GRP + wi
    cell = (core * NCELLS + cell_l).astype(np.int64)
    order = np.lexsort((rel, cell))
    cell_s = cell[order]
    rel_s = rel[order]
    dw_s = dst_in_win[order]

    counts = np.bincount(cell_s, minlength=NCORES * NCELLS)
    starts = np.zeros(NCORES * NCELLS + 1, np.int64)
    np.cumsum(counts, out=starts[1:])
    pos = np.arange(E) - starts[cell_s]

    counts_pc = counts.reshape(NCORES, NCELLS)
    nvalid = np.maximum(counts_pc.max(axis=0), 1).astype(np.int64)  # [NCELLS]
    slots_h = int(math.ceil(nvalid.max() / 128.0) * 128)
    tot = NCELLS * slots_h

    # fake slots gather row 0 (valid data) and carry dloc=-1 (zero one-hot
    # row), so every slot is gathered and no memset/clipping is needed.
    idx_slots = np.zeros((NCORES, NCELLS, slots_h), np.int16)
    dloc_slots = np.full((NCORES, NCELLS, slots_h), -1.0, np.float32)
    c_s = cell_s // NCELLS
    l_s = cell_s % NCELLS
    idx_slots[c_s, l_s, pos] = rel_s.astype(np.int16)
    dloc_slots[c_s, l_s, pos] = dw_s

    idx_tiles, dloc_tiles = [], []
    for c in range(NCORES):
        it = idx_slots[c].reshape(tot // 16, 16).T  # slot j at [j%16, j//16]
        idx_tiles.append(np.tile(it, (8, 1)).copy())
        dloc_tiles.append(
            dloc_slots[c].reshape(tot // 128, 128).T.astype(np.float32).copy()
        )

    return norm, idx_tiles, dloc_tiles, nvalid, slots_h


def _regionize(x):
    """[N, D] node-ordered -> [REGA+REGB, D] region layout."""
    out = np.zeros((REGA + REGB, x.shape[1]), np.float32)
    for r in range(NCORES):
        sh = x[r * SHARD : (r + 1) * SHARD]
        out[r * ROWSA : r * ROWSA + min(ROWSA, sh.shape[0])] = sh[:ROWSA]
        nb = max(SHARD - ROWSA, 0)
        if nb:
            out[REGA + r * ROWSB : REGA + r * ROWSB + nb] = sh[ROWSA:]
    return out


def _build(slots_h, nvalid):
    """Build the 8-core SPMD Bass program (same program on every core)."""
    nc = bacc.Bacc(
        "TRN2",
        target_bir_lowering=False,
        debug=False,
        num_devices=NCORES,
        num_swdge_queues=NQUEUES,
        dynamic_dma_scratch_size=SCRATCH,
    )

    tot = NCELLS * slots_h
    nbw = slots_h // 128          # blocks per (window, region) cell
    nbg = GRP * nbw               # blocks per (group, region) gather

    feat_full_p = nc.declare_dram_parameter(
        "feat_full", [REGA + REGB, D], F32, isOutput=False
    )
    featT_p = nc.declare_dram_parameter("featT", [D, SHARD_PAD], BF16, isOutput=False)
    idx_p = nc.declare_dram_parameter("idx", [128, tot // 16], I16, isOutput=False)
    dloc_p = nc.declare_dram_parameter("dloc", [128, tot // 128], BF16, isOutput=False)
    ncol_p = nc.declare_dram_parameter("ncol", [128, NWIN], F32, isOutput=False)
    ncol2_p = nc.declare_dram_parameter("ncol2", [128, NWIN], F32, isOutput=False)
    wt_p = nc.declare_dram_parameter("wt", [D, 4 * D], BF16, isOutput=False)
    bias_p = nc.declare_dram_parameter("bias", [128, D], F32, isOutput=False)
    iota_p = nc.declare_dram_parameter("iota", [128, 128], BF16, isOutput=False)
    ident_p = nc.declare_dram_parameter("ident", [128, 128], BF16, isOutput=False)
    out_p = nc.declare_dram_parameter("out", [SHARD_PAD, D], F32, isOutput=True)

    with tile.TileContext(nc) as tc:
        with (
            tc.tile_pool(name="meta", bufs=1) as meta,
            tc.tile_pool(name="gpool", bufs=3) as gpool,
            tc.tile_pool(name="gbpool", bufs=3) as gbpool,
            tc.tile_pool(name="spool", bufs=2) as spool,
            tc.tile_pool(name="work", bufs=3) as work,
            tc.tile_pool(name="hstore", bufs=1) as hstore_pool,
            tc.tile_pool(name="ps", bufs=4, space="PSUM") as ps_pool,
            tc.tile_pool(name="pso", bufs=2, space="PSUM") as pso_pool,
            tc.tile_pool(name="pst", bufs=2, space="PSUM") as pst_pool,
            tc.tile_pool(name="dram", bufs=1, space="DRAM") as dram,
        ):
            # ---- metadata preload (resident in SBUF); idx first so hop-1
            # gathers can start as soon as it lands ----
            idx_sb = meta.tile([128, tot // 16], I16)
            nc.sync.dma_start(idx_sb[:], idx_p[:])
            dloc_sb = meta.tile([128, tot // 128], BF16)
            nc.sync.dma_start(dloc_sb[:], dloc_p[:])
            ncol_sb = meta.tile([128, NWIN], F32)
            nc.sync.dma_start(ncol_sb[:], ncol_p[:])
            ncol2_sb = meta.tile([128, NWIN], F32)
            nc.sync.dma_start(ncol2_sb[:], ncol2_p[:])
            wt_sb = meta.tile([D, 4 * D], BF16)
            nc.sync.dma_start(wt_sb[:], wt_p[:])
            bias_sb = meta.tile([128, D], F32)
            nc.sync.dma_start(bias_sb[:], bias_p[:])
            iota_sb = meta.tile([128, 128], BF16)
            nc.sync.dma_start(iota_sb[:], iota_p[:])
            ident_sb = meta.tile([128, 128], BF16)
            nc.sync.dma_start(ident_sb[:], ident_p[:])
            featT_sb = meta.tile([D, SHARD_PAD], BF16)
            nc.sync.dma_start(featT_sb[:], featT_p[:])

            # h_k tiles for k=1,2 kept for the final linear (bf16)
            hstore = hstore_pool.tile([128, (HOPS - 1) * NWIN * D], BF16)

            for _rep in range(REPS):
                # per-hop-boundary A/B exchange buffers (hop 1 and 2 outputs);
                # Shared DRAM wants a single writer, so allocate per rep.
                aspace = "Shared" if SHARED_AG else "Local"
                g_inA = [
                    dram.tile([ROWSA, D], F32, name=f"g_inA{k}_{_rep}")
                    for k in range(2)
                ]
                g_inB = [
                    dram.tile([ROWSB, D], F32, name=f"g_inB{k}_{_rep}")
                    for k in range(2)
                ]
                g_fullA = [
                    dram.tile(
                        [REGA, D], F32, name=f"g_fullA{k}_{_rep}", addr_space=aspace
                    )
                    for k in range(2)
                ]
                g_fullB = [
                    dram.tile(
                        [REGB, D], F32, name=f"g_fullB{k}_{_rep}", addr_space=aspace
                    )
                    for k in range(2)
                ]
                _phases(
                    nc, slots_h, feat_full_p, out_p,
                    g_inA, g_inB, g_fullA, g_fullB,
                    idx_sb, dloc_sb, ncol_sb, ncol2_sb, wt_sb, bias_sb,
                    iota_sb, ident_sb, featT_sb, hstore,
                    gpool, gbpool, spool, work, ps_pool, pso_pool, pst_pool,
                )
    nc.compile()
    return nc


AG_ENGINE = "gpsimd"  # engine issuing the collective doorbell


def _ag(nc, g_in, g_full):
    if NO_COLLECTIVE:
        nc.sync.dma_start(g_full[0 : g_in.shape[0], :], g_in[:, :])
    else:
        eng = getattr(nc, AG_ENGINE)
        eng.collective_compute(
            "AllGather",
            mybir.AluOpType.bypass,
            replica_groups=[list(range(NCORES))],
            ins=[g_in.opt()],
            outs=[g_full.opt()],
        )


def _phases(
    nc, slots_h, feat_full_p, out_p,
    g_inA, g_inB, g_fullA, g_fullB,
    idx_sb, dloc_sb, ncol_sb, ncol2_sb, wt_sb, bias_sb,
    iota_sb, ident_sb, featT_sb, hstore,
    gpool, gbpool, spool, work, ps_pool, pso_pool, pst_pool,
):
    nbw = slots_h // 128          # blocks per cell
    nbg = GRP * nbw               # blocks per (group, region)
    for k in range(1, HOPS + 1):
        if k == 1:
            srcA = feat_full_p[0:REGA, :]
            srcB = feat_full_p[REGA : REGA + REGB, :]
        else:
            srcA = g_fullA[k - 2][:, :]
            srcB = g_fullB[k - 2][:, :]
        for g in range(NGRP):
            gts = []
            gbs = []
            sts = []
            for r in range(2):
                cell0 = g * (2 * GRP) + r * GRP
                blk0 = cell0 * nbw
                # ---- one gather for GRP windows x one source region ----
                gt = gpool.tile([128, nbg, D], F32, tag="G", name=f"G{g}_{r}")
                if not NO_GATHERS:
                    tot_idx = GRP * slots_h
                    col0 = cell0 * (slots_h // 16)
                    ch = GCH if GCH else tot_idx
                    ch = (ch // 128) * 128
                    for j0 in range(0, tot_idx, ch):
                        nidx = min(ch, tot_idx - j0)
                        nc.gpsimd.dma_gather(
                            gt[:, j0 // 128 : (j0 + nidx) // 128, :],
                            srcA if r == 0 else srcB,
                            idx_sb[:, col0 + j0 // 16 : col0 + (j0 + nidx) // 16],
                            nidx,
                            nidx,
                            D,
                            elem_step=D,
                            single_packet=SINGLE_PACKET,
                            queue_num=(g * 2 + r) % NQUEUES,
                        )
                # ---- bf16 conversion of the gathered rows (ACT) ----
                gb = gbpool.tile([128, nbg, D], BF16, tag="GB", name=f"GB{g}_{r}")
                nc.scalar.activation(
                    gb[:, :, :],
                    gt[:, :, :],
                    mybir.ActivationFunctionType.Copy,
                )
                # ---- batched one-hot build: S[p, b, v] = (iota[v]==dloc[p,b]) ----
                st = spool.tile([128, nbg, 128], BF16, tag="S", name=f"S{g}_{r}")
                nc.vector.tensor_tensor(
                    st[:, :, :],
                    iota_sb[:].unsqueeze(1).broadcast_to([128, nbg, 128]),
                    dloc_sb[:, blk0 : blk0 + nbg].unsqueeze(2).broadcast_to(
                        [128, nbg, 128]
                    ),
                    mybir.AluOpType.is_equal,
                )
                gts.append(gt)
                gbs.append(gb)
                sts.append(st)
            for wi in range(GRP):
                w = g * GRP + wi
                ps = ps_pool.tile([128, D], F32)
                for r in range(2):
                    for b in range(nbw):
                        nc.tensor.matmul(
                            ps[:],
                            sts[r][:, wi * nbw + b, :],
                            gbs[r][:, wi * nbw + b, :],
                            start=(r == 0 and b == 0),
                            stop=(r == 1 and b == nbw - 1),
                        )
                # ps holds raw segment-sum s_k for window w
                if k < HOPS:
                    hslice = hstore[
                        :, ((k - 1) * NWIN + w) * D : ((k - 1) * NWIN + w + 1) * D
                    ]
                    # h_k = s * norm  (bf16, kept for the final linear)
                    nc.vector.tensor_scalar_mul(hslice, ps[:], ncol_sb[:, w : w + 1])
                    # g_k = s * norm^2 (f32, exchanged for next hop's gathers)
                    gsb = work.tile([128, D], F32, tag="gsb")
                    nc.scalar.activation(
                        gsb[:],
                        ps[:],
                        mybir.ActivationFunctionType.Copy,
                        scale=ncol2_sb[:, w : w + 1],
                    )
                    if w < WA:
                        nc.sync.dma_start(
                            g_inA[k - 1][w * 128 : (w + 1) * 128, :], gsb[:]
                        )
                    else:
                        nc.sync.dma_start(
                            g_inB[k - 1][(w - WA) * 128 : (w - WA + 1) * 128, :],
                            gsb[:],
                        )
                    if w == WA - 1:
                        # region A complete: exchange it while B still computes
                        _ag(nc, g_inA[k - 1], g_fullA[k - 1])
                else:
                    # final linear for window w
                    po = pso_pool.tile([128, D], F32)
                    nc.tensor.matmul(
                        po[:],
                        featT_sb[:, w * 128 : (w + 1) * 128],
                        wt_sb[:, 0:D],
                        start=True,
                        stop=False,
                    )
                    for kk in range(1, HOPS + 1):
                        if kk < HOPS:
                            hsrc = hstore[
                                :,
                                ((kk - 1) * NWIN + w) * D : ((kk - 1) * NWIN + w + 1)
                                * D,
                            ]
                        else:
                            h3 = work.tile([128, D], BF16, tag="h3")
                            nc.vector.tensor_scalar_mul(
                                h3[:], ps[:], ncol_sb[:, w : w + 1]
                            )
                            hsrc = h3[:]
                        pt = pst_pool.tile([D, 128], BF16)
                        nc.tensor.matmul(pt[:], hsrc, ident_sb[:], is_transpose=True)
                        hT = work.tile([D, 128], BF16, tag="hT")
                        nc.vector.tensor_copy(hT[:], pt[:])
                        nc.tensor.matmul(
                            po[:],
                            hT[:],
                            wt_sb[:, kk * D : (kk + 1) * D],
                            start=False,
                            stop=(kk == HOPS),
                        )
                    osb = work.tile([128, D], F32, tag="osb")
                    nc.vector.tensor_add(osb[:], po[:], bias_sb[:])
                    nc.sync.dma_start(out_p[w * 128 : (w + 1) * 128, :], osb[:])
        if k < HOPS:
            _ag(nc, g_inB[k - 1], g_fullB[k - 1])


def _make_in_maps(feat, src, dst, W, b):
    feat = np.ascontiguousarray(np.asarray(feat), dtype=np.float32)
    W = np.ascontiguousarray(np.asarray(W), dtype=np.float32)
    b = np.ascontiguousarray(np.asarray(b), dtype=np.float32)

    norm, idx_tiles, dloc_tiles, nvalid, slots_h = _preprocess(src, dst)

    # host-prescaled gather table: g_0 = norm * feat
    feat_full = _regionize(norm[:, None] * feat)
    wt = np.concatenate(
        [W[:, k * D : (k + 1) * D].T for k in range(HOPS + 1)], axis=1
    ).astype(np.float32)
    bias = np.tile(b[None, :], (128, 1)).copy()
    iota = np.tile(np.arange(128, dtype=np.float32)[None, :], (128, 1))
    ident = np.eye(128, dtype=np.float32)

    def bf(x):
        import jax.numpy as jnp

        return np.asarray(jnp.asarray(x, dtype=jnp.bfloat16))

    in_maps = []
    for c in range(NCORES):
        fs = np.zeros((SHARD_PAD, D), np.float32)
        fs[:SHARD] = feat[c * SHARD : (c + 1) * SHARD]
        ns = np.zeros(SHARD_PAD, np.float32)
        ns[:SHARD] = norm[c * SHARD : (c + 1) * SHARD]
        in_maps.append(
            {
                "feat_full": feat_full,
                "featT": bf(fs.T.copy()),
                "idx": idx_tiles[c],
                "dloc": bf(dloc_tiles[c]),
                "ncol": np.ascontiguousarray(ns.reshape(NWIN, 128).T),
                "ncol2": np.ascontiguousarray((ns * ns).reshape(NWIN, 128).T),
                "wt": bf(wt),
                "bias": bias,
                "iota": bf(iota),
                "ident": bf(ident),
            }
        )
    return in_maps, nvalid, slots_h


def _run(feat, src, dst, W, b, trace=False):
    in_maps, nvalid, slots_h = _make_in_maps(feat, src, dst, W, b)
    nc = _build(slots_h, nvalid)
    res = run_bass_kernel_spmd(nc, in_maps, list(range(NCORES)), trace=trace)
    out = np.concatenate(
        [res.results[c]["out"][:SHARD] for c in range(NCORES)], axis=0
    )
    return out, res


def kernel(feat, src, dst, W, b):
    out, _ = _run(feat, src, dst, W, b, trace=False)
    return out


def kernel_traced(feat, src, dst, W, b):
    return _run(feat, src, dst, W, b, trace=True)
